# revision 46
# baseline (speedup 1.0000x reference)
"""ArcFace head on 8 TRN2 NeuronCores — transposed class-parallel layout.

Core c owns classes [c*12500, (c+1)*12500). The cos matmul runs with classes
on the PSUM partition axis: out[c_local, b] = S * (F_hat_b . W_c) * inv|W_c|,
so the post-matmul scale is a per-partition scalar applied by the Act engine
during PSUM->SBUF eviction.

DMA structure (v2 redesign): all weight traffic is bundled into 13 one-piece
DMAs (1 MB each, 3D access pattern covering all four 128-row D-chunks of a
1024-class column range) and issued up-front on the Sync HWDGE queue with no
interleaved semaphore waits, so the SDMA rings always have weight descriptors
queued ahead of demand. Feature/label-weight inputs and all output DMAs ride
the Scalar HWDGE queue (a separate ring the SDMA round-robins with), so
output traffic can never starve the weight prefetch and output staging
recycles promptly.

Features are pre-normalized on device (S/|f| folded into the moving operand).
The ArcFace margin values for every row are computed on device in a
replicated-row layout (no per-row gather needed) and written to a small vout
tensor; the host applies them to the target columns by fancy indexing
(indexing only, no host FLOPs).

Everything on device runs in bf16 (full PE rate), with all reductions
accumulating in fp32 PSUM.
"""

import math
import os

import numpy as np

B = 512
D = 512
C = 100000
NCORES = 8
CS = C // NCORES          # 12500 classes per core

M_MARGIN = 0.5
S_SCALE = 64.0
TH = math.cos(math.pi - M_MARGIN)
MM_ = math.sin(math.pi - M_MARGIN) * M_MARGIN

P = 128
NK = D // P               # 4 contraction chunks
NCH = (CS + P - 1) // P   # 98 class chunks (97*128 + 84)
PW = 1024                 # classes per piece (8 chunks)
NPC = (CS + PW - 1) // PW  # 13 pieces, last = 212 wide

_CACHE = {}


def _pieces():
    out = []
    for p in range(NPC):
        lo = p * PW
        hi = min(CS, lo + PW)
        out.append((lo, hi))
    return out


def _build_nc(opts=None):
    opts = opts or {}
    import concourse.tile as tile
    from concourse import bacc, mybir

    dt = mybir.dt
    Alu = mybir.AluOpType
    Act = mybir.ActivationFunctionType

    nc = bacc.Bacc("TRN2", target_bir_lowering=False, debug=False,
                   enable_asserts=False, num_devices=NCORES)

    wt = nc.dram_tensor("wt", [D, CS], dt.bfloat16, kind="ExternalInput").ap()
    featT = nc.dram_tensor("featT", [D, B], dt.bfloat16,
                           kind="ExternalInput").ap()
    wgT = nc.dram_tensor("wgT", [D, B], dt.bfloat16, kind="ExternalInput").ap()
    out = nc.dram_tensor("out", [CS, B], dt.bfloat16, kind="ExternalOutput").ap()
    vout = nc.dram_tensor("vout", [1, B], dt.float32,
                          kind="ExternalOutput").ap()

    ev_eng = opts.get("ev_eng", "aav")    # eviction engine cycle (per chunk)
    sq_eng = opts.get("sq_eng", "v")      # engine cycle for piece squares
    JB = opts.get("jb", 8)                # chunks batched per output DMA
    pieces = _pieces()

    with tile.TileContext(nc) as tc:
        with (
            tc.tile_pool(name="const", bufs=1) as constp,
            tc.tile_pool(name="fm", bufs=2) as fmp,      # feature/margin scratch
            tc.tile_pool(name="sqs", bufs=2) as sqp,     # square scratch
            tc.tile_pool(name="outp", bufs=opts.get("osb_bufs", 4)) as outp,
            tc.tile_pool(name="ps_o", bufs=opts.get("ps_o", 6),
                         space="PSUM") as ps_o,
            tc.tile_pool(name="ps_n", bufs=1, space="PSUM") as ps_n,
            tc.tile_pool(name="ps_f", bufs=1, space="PSUM") as ps_f,
        ):
            # ---- constants ----
            ones_sq = constp.tile([P, P], dt.bfloat16, tag="ones_sq")
            nc.vector.memset(ones_sq[:], 1.0)
            ones_col = constp.tile([P, 1], dt.bfloat16, tag="ones_col")
            nc.vector.memset(ones_col[:], 1.0)

            # ---- Act table preload: touch every activation function used
            # later so the ~1.5us ACT_TABLE_LOADs happen during boot idle ----
            actwarm = constp.tile([P, 2], dt.bfloat16, tag="actwarm")
            nc.scalar.activation(actwarm[:, 0:1], ones_col[:], Act.Square,
                                 bias=0.0, scale=1.0)
            nc.scalar.activation(actwarm[:, 1:2], ones_col[:],
                                 Act.Abs_reciprocal_sqrt, bias=0.0, scale=1.0)

            # ---- PE pstate warmup: dummy matmuls while DMAs land ----
            nwarm = opts.get("nwarm", 20)
            if nwarm:
                warm = ps_f.tile([P, B], dt.float32, tag="psf")
                for _ in range(nwarm):
                    nc.tensor.matmul(warm[:, 0:P], ones_sq[:], ones_sq[:],
                                     start=True, stop=True)

            # ---- input DMAs, all on the Sync HWDGE queue, issued up-front
            # with no interleaved waits. featT + piece 0 go first at per-k
            # granularity so the boot chain pipelines; later pieces are one
            # bundled 3D DMA each. ----
            fT = constp.tile([P, NK * B], dt.bfloat16, tag="fT")
            for k in range(NK):
                nc.sync.dma_start(fT[:, k * B:(k + 1) * B],
                                  featT[k * P:(k + 1) * P, :])

            # Weight pieces live in four per-k rotating 6-slot pools (one DMA
            # per (piece, k); separate pools keep the four stationary tiles
            # of a chunk far apart in SBUF, matching the address phasing the
            # PE LDWEIGHTS stream runs fastest with). DMAs run 4-5 pieces
            # ahead of consumption, so slot reuse never has to wait.
            wtps = {}

            def emit_wt_dma(p, halves=1):
                lo, hi = pieces[p]
                tiles = []
                for k in range(NK):
                    wtpk = sqp.tile([P, PW], dt.bfloat16, tag=f"wtp{k}",
                                    bufs=6)
                    tiles.append(wtpk)
                hw = (hi - lo) // halves
                for h in range(halves):
                    for k in range(NK):
                        nc.sync.dma_start(
                            tiles[k][:, h * hw:(h + 1) * hw],
                            wt[k * P:(k + 1) * P,
                               lo + h * hw:lo + (h + 1) * hw])
                wtps[p] = tiles

            # first two pieces arrive in half-piece waves so the first
            # matmul chunks unblock sooner at boot
            emit_wt_dma(0, halves=2)
            emit_wt_dma(1, halves=2)
            for p in range(2, 5):
                emit_wt_dma(p)
            wG = constp.tile([P, NK * B], dt.bfloat16, tag="wG")
            nc.sync.dma_start(
                wG[:, :].rearrange("p (k b) -> p k b", k=NK),
                wgT[:, :].rearrange("(k p) b -> p k b", k=NK, p=P))

            # ---- feature normalization: fhat = S * f / |f|, bf16;
            # per-k so the chain pipelines with the fT chunk DMAs ----
            sqf = fmp.tile([P, NK * B], dt.bfloat16, tag="sqf", bufs=1)
            ssf = ps_f.tile([P, B], dt.float32, tag="psf")
            for k in range(NK):
                nc.vector.tensor_mul(sqf[:, k * B:(k + 1) * B],
                                     fT[:, k * B:(k + 1) * B],
                                     fT[:, k * B:(k + 1) * B])
            for k in range(NK):
                nc.tensor.matmul(ssf[:], ones_sq[:], sqf[:, k * B:(k + 1) * B],
                                 start=(k == 0), stop=(k == NK - 1))
            invfS = constp.tile([P, B], dt.bfloat16, tag="invfS")
            # rsqrt(ssf / S^2) = S / |f|
            nc.scalar.activation(invfS[:], ssf[:], Act.Abs_reciprocal_sqrt,
                                 bias=0.0, scale=1.0 / (S_SCALE * S_SCALE))
            fhat = constp.tile([P, NK * B], dt.bfloat16, tag="fhat")
            for k in range(NK):
                nc.vector.tensor_tensor(out=fhat[:, k * B:(k + 1) * B],
                                        in0=fT[:, k * B:(k + 1) * B],
                                        in1=invfS[:], op=Alu.mult)

            # ---- resident norm scale tile ----
            invw = constp.tile([P, NCH], dt.float32, tag="invw")

            def emit_chain(p):
                # squared weights + k-reduction for one piece
                lo, hi = pieces[p]
                w = hi - lo
                sq = sqp.tile([P, NK * PW], dt.bfloat16, tag="sq", bufs=2)
                t1 = sqp.tile([P, 2 * PW], dt.bfloat16, tag="t1", bufs=2)
                pp = sqp.tile([P, PW], dt.bfloat16, tag="pp", bufs=2)
                e = sq_eng[p % len(sq_eng)]
                for k in range(NK):
                    ksl = slice(k * w, (k + 1) * w)
                    src = wtps[p][k][:, 0:w]
                    if (e == "a") != (p == 0 and k % 2 == 1):
                        nc.scalar.activation(sq[:, ksl], src, Act.Square,
                                             bias=0.0, scale=1.0)
                    else:
                        nc.vector.tensor_mul(sq[:, ksl], src, src)
                nc.vector.tensor_tensor(out=t1[:, 0:2 * w],
                                        in0=sq[:, 0:2 * w],
                                        in1=sq[:, 2 * w:4 * w], op=Alu.add)
                nc.vector.tensor_tensor(out=pp[:, 0:w], in0=t1[:, 0:w],
                                        in1=t1[:, w:2 * w], op=Alu.add)
                return pp

            # chunk lists per piece: (global chunk id, col-in-piece, width)
            chunks_of = []
            gfirst = 0
            for (lo, hi) in pieces:
                glast = min(NCH, (hi + P - 1) // P)
                chunks_of.append([(g, g * P - lo, min(P, CS - g * P))
                                  for g in range(gfirst, glast)])
                gfirst = glast

            def emit_norms(p, pp):
                # class norms: one 1-col matmul per 128-class chunk, then the
                # per-partition inv-norm via Act rsqrt into resident invw
                chs = chunks_of[p]
                nch = len(chs)
                g0 = chs[0][0]
                nps = ps_n.tile([P, 8], dt.float32, tag="nps")
                for i, (g, c0, cw) in enumerate(chs):
                    nc.tensor.matmul(nps[0:cw, i:i + 1],
                                     pp[:, c0:c0 + cw], ones_col[:],
                                     start=True, stop=True)
                cwl = chs[-1][2]
                if cwl == P:
                    nc.scalar.activation(invw[:, g0:g0 + nch], nps[:, 0:nch],
                                         Act.Abs_reciprocal_sqrt,
                                         bias=0.0, scale=1.0)
                else:  # last chunk is 84 classes; avoid unwritten PSUM rows
                    if nch > 1:
                        nc.scalar.activation(invw[:, g0:g0 + nch - 1],
                                             nps[:, 0:nch - 1],
                                             Act.Abs_reciprocal_sqrt,
                                             bias=0.0, scale=1.0)
                    nc.scalar.activation(invw[0:cwl, g0 + nch - 1:g0 + nch],
                                         nps[0:cwl, nch - 1:nch],
                                         Act.Abs_reciprocal_sqrt,
                                         bias=0.0, scale=1.0)

            mtiles = {}

            def emit_margin_pre():
                # DVE products for the margin reductions, emitted well before
                # the margin matmuls so the tensor queue never waits on them
                sqwg = fmp.tile([P, NK * B], dt.bfloat16, tag="sqwg", bufs=1)
                nc.vector.tensor_mul(sqwg[:], wG[:], wG[:])
                fg = fmp.tile([P, NK * B], dt.bfloat16, tag="fg", bufs=1)
                nc.vector.tensor_mul(fg[:], fhat[:], wG[:])
                mtiles["sqwg"] = sqwg
                mtiles["fg"] = fg

            def emit_margin():
                """ArcFace margin: reductions + trig; inputs already in SBUF
                so the tensor-queue matmuls never stall."""
                sqwg = mtiles["sqwg"]
                fg = mtiles["fg"]
                sswg = ps_f.tile([P, B], dt.float32, tag="psf")
                for k in range(NK):
                    nc.tensor.matmul(sswg[:], ones_sq[:],
                                     sqwg[:, k * B:(k + 1) * B],
                                     start=(k == 0), stop=(k == NK - 1))
                # rsqrt(sswg * S^2) = 1 / (S * |wg|)
                invwg = fmp.tile([1, B], dt.bfloat16, tag="invwg", bufs=1)
                nc.scalar.activation(invwg[:], sswg[0:1, :],
                                     Act.Abs_reciprocal_sqrt,
                                     bias=0.0, scale=S_SCALE * S_SCALE)
                dot = ps_f.tile([P, B], dt.float32, tag="psf")
                for k in range(NK):
                    nc.tensor.matmul(dot[:], ones_sq[:],
                                     fg[:, k * B:(k + 1) * B],
                                     start=(k == 0), stop=(k == NK - 1))
                # t = cos(theta); the reductions are partition-replicated,
                # so the scalar chain runs on single-partition [1, B] slices
                t = fmp.tile([1, B], dt.bfloat16, tag="t", bufs=1)
                nc.vector.tensor_tensor(out=t[:], in0=dot[0:1, :],
                                        in1=invwg[:], op=Alu.mult)
                nc.vector.tensor_scalar_min(t[:], t[:], 1.0)
                nc.vector.tensor_scalar_max(t[:], t[:], -1.0)
                om = fmp.tile([1, B], dt.bfloat16, tag="om", bufs=1)
                nc.vector.tensor_mul(om[:], t[:], t[:])
                nc.vector.tensor_scalar(out=om[:], in0=om[:], scalar1=-1.0,
                                        scalar2=1.0, op0=Alu.mult, op1=Alu.add)
                rs = fmp.tile([1, B], dt.bfloat16, tag="rs", bufs=1)
                nc.scalar.activation(rs[:], om[:], Act.Abs_reciprocal_sqrt,
                                     bias=0.0, scale=1.0)
                r = fmp.tile([1, B], dt.bfloat16, tag="r", bufs=1)
                nc.vector.tensor_mul(r[:], om[:], rs[:])   # sqrt(1 - t^2)
                a1 = fmp.tile([1, B], dt.bfloat16, tag="a1", bufs=1)
                nc.vector.tensor_scalar_mul(a1[:], t[:], math.cos(M_MARGIN))
                a2 = fmp.tile([1, B], dt.bfloat16, tag="a2", bufs=1)
                nc.vector.tensor_scalar_mul(a2[:], r[:], math.sin(M_MARGIN))
                adjA = fmp.tile([1, B], dt.bfloat16, tag="adjA", bufs=1)
                nc.vector.tensor_tensor(out=adjA[:], in0=a1[:], in1=a2[:],
                                        op=Alu.subtract)
                mask = fmp.tile([1, B], dt.int8, tag="mask", bufs=1)
                nc.vector.tensor_scalar(out=mask[:], in0=t[:], scalar1=TH,
                                        scalar2=None, op0=Alu.is_gt)
                adj = fmp.tile([1, B], dt.bfloat16, tag="adj", bufs=1)
                nc.vector.tensor_scalar_sub(adj[:], t[:], MM_)
                nc.vector.copy_predicated(adj[:], mask[:], adjA[:])
                val = fmp.tile([1, B], dt.float32, tag="val", bufs=1)
                nc.vector.tensor_scalar_mul(val[:], adj[:], S_SCALE)
                nc.scalar.dma_start(vout[:], val[0:1, :])

            def emit_batch_mms(p, i, nb):
                # PE matmuls for one output batch of nb chunks
                lo, hi = pieces[p]
                w = hi - lo
                chs = chunks_of[p]
                wtp = wtps[p]
                pos = []
                for j in range(nb):
                    g, c0, cw = chs[i + j]
                    po = ps_o.tile([P, B], dt.float32, tag="po")
                    for k in range(NK):
                        nc.tensor.matmul(
                            po[0:cw, :],
                            wtp[k][:, c0:c0 + cw],
                            fhat[:, k * B:(k + 1) * B],
                            start=(k == 0), stop=(k == NK - 1))
                    pos.append(po)
                return pos

            def emit_batch_out(p, i, pos):
                # evictions (scale-copy, spread over Act/DVE/GpSimd per
                # ev_eng) + one batched out DMA on the Scalar HWDGE queue
                lo, hi = pieces[p]
                chs = chunks_of[p]
                nb = len(pos)
                full = all(chs[i + j][2] == P for j in range(nb))
                osb = outp.tile([P, nb * B], dt.bfloat16, tag=f"osb{nb}",
                                bufs=opts.get("osb_bufs", 6) if nb == JB
                                else 2)
                for j in range(nb):
                    g, c0, cw = chs[i + j]
                    osl = osb[0:cw, j * B:j * B + B]
                    # the final piece's two evictions run on different
                    # engines so they finish in parallel at the tail
                    e = ("av"[j % 2] if p == NPC - 1
                         else ev_eng[g % len(ev_eng)])
                    if e == "v":
                        nc.vector.tensor_scalar_mul(osl, pos[j][0:cw, :],
                                                    invw[0:cw, g:g + 1])
                    else:
                        nc.scalar.activation(osl, pos[j][0:cw, :], Act.Copy,
                                             bias=0.0,
                                             scale=invw[0:cw, g:g + 1])
                c0 = chs[i][1] + lo
                # the last (small) piece drains on the otherwise-idle Scalar
                # HWDGE ring; everything else on Sync. Keeping the big
                # second-to-last batches off the Act queue matters: a 2.4us
                # DMA issue there would delay the final evictions
                deng = nc.scalar if p == NPC - 1 else nc.sync
                if full:
                    dst = out[c0:c0 + nb * P, :].rearrange(
                        "(j p) b -> p j b", j=nb, p=P)
                    src = osb[:, 0:nb * B].rearrange("p (j b) -> p j b", j=nb)
                    deng.dma_start(dst, src)
                else:
                    for j in range(nb):
                        g, cj, cw = chs[i + j]
                        dj = nc.scalar if (p == NPC - 1 and j % 2 == 0) \
                            else nc.sync
                        dj.dma_start(
                            out[lo + cj:lo + cj + cw, :],
                            osb[0:cw, j * B:j * B + B])

            def emit_mains(p, jb=JB, i0=0):
                chs = chunks_of[p]
                i = i0
                while i < len(chs):
                    nb = min(jb, len(chs) - i)
                    pos = emit_batch_mms(p, i, nb)
                    emit_batch_out(p, i, pos)
                    i += nb

            # ---- software pipeline over class pieces ----
            # Piece 0 is special: the first 4 chunks' matmuls are emitted
            # ahead of the norm matmuls (PE starts as soon as fhat + piece 0
            # are in SBUF) while their evictions queue behind the invw rsqrt.
            pps = {0: emit_chain(0), 1: emit_chain(1)}
            pos0 = emit_batch_mms(0, 0, 4)
            emit_norms(0, pps[0])
            emit_batch_out(0, 0, pos0)
            emit_mains(0, i0=4)
            emit_norms(1, pps[1])
            for p in range(1, NPC):
                if p + 4 < NPC:
                    emit_wt_dma(p + 4)
                if p + 1 < NPC:
                    pps[p + 1] = emit_chain(p + 1)
                if p == 1:
                    emit_margin_pre()
                if p == 4:
                    emit_margin()
                # the last full piece drains in 4-chunk batches so its out
                # DMAs overlap the final evictions instead of one big issue
                emit_mains(p, jb=4 if p == NPC - 2 else JB)
                if p + 1 < NPC:
                    emit_norms(p + 1, pps[p + 1])

    nc.compile()
    return nc


def _get_nc(opts=None):
    key = tuple(sorted((opts or {}).items()))
    if key not in _CACHE:
        _CACHE[key] = _build_nc(opts)
    return _CACHE[key]


def _enable_trace_hook():
    import sys
    import types
    try:
        import antenv.axon_hooks  # noqa: F401
        return
    except ImportError:
        pass
    import antenv
    mod = types.ModuleType("antenv.axon_hooks")
    holder = [None]
    mod.set_axon_ntff_profile_hook = lambda h: holder.__setitem__(0, h)
    mod.get_axon_ntff_profile_hook = lambda: holder[0]
    sys.modules["antenv.axon_hooks"] = mod
    antenv.axon_hooks = mod
    try:
        from trn_agent_boot.trn_boot import _ntff_profile_via_ctypes
        mod.set_axon_ntff_profile_hook(
            _ntff_profile_via_ctypes("/opt/axon/libaxon_pjrt.so"))
    except Exception:
        pass


LAST_EXEC_NS = None
LAST_RESULTS = None
_OPTS = {}


def kernel(features, labels, weight):
    global LAST_EXEC_NS, LAST_RESULTS
    import ml_dtypes
    from concourse.bass_utils import run_bass_kernel_spmd

    features = np.asarray(features)
    weight = np.asarray(weight)
    labels = np.asarray(labels).astype(np.int64)

    trace = bool(int(os.environ.get("ARCFACE_TRACE", "0")))
    if trace:
        _enable_trace_hook()

    nc = _get_nc(_OPTS.get("opts"))

    featT_np = np.ascontiguousarray(features.T.astype(ml_dtypes.bfloat16))
    wgT_np = np.ascontiguousarray(weight[labels].T.astype(ml_dtypes.bfloat16))
    wt16 = weight.astype(ml_dtypes.bfloat16)

    in_maps = []
    for c in range(NCORES):
        c0 = c * CS
        wt_c = np.ascontiguousarray(wt16[c0:c0 + CS].T)  # [D, CS] bf16
        in_maps.append({
            "wt": wt_c,
            "featT": featT_np,
            "wgT": wgT_np,
        })

    res = run_bass_kernel_spmd(nc, in_maps, core_ids=list(range(NCORES)),
                               trace=trace)
    LAST_EXEC_NS = res.exec_time_ns
    LAST_RESULTS = res

    full = np.empty((B, C), dtype=np.float32)
    for c in range(NCORES):
        full[:, c * CS:(c + 1) * CS] = res.results[c]["out"].T
    rows = np.arange(B)
    full[rows, labels] = np.asarray(res.results[0]["vout"]).reshape(B)
    return full


# revision 47
# speedup vs baseline: 1.0298x; 1.0298x over previous
"""ArcFace head on 8 TRN2 NeuronCores — transposed class-parallel layout.

Core c owns classes [c*12500, (c+1)*12500). The cos matmul runs with classes
on the PSUM partition axis: out[c_local, b] = S * (F_hat_b . W_c) * inv|W_c|,
so the post-matmul scale is a per-partition scalar applied by the Act engine
during PSUM->SBUF eviction.

DMA structure (v2 redesign): all weight traffic is bundled into 13 one-piece
DMAs (1 MB each, 3D access pattern covering all four 128-row D-chunks of a
1024-class column range) and issued up-front on the Sync HWDGE queue with no
interleaved semaphore waits, so the SDMA rings always have weight descriptors
queued ahead of demand. Feature/label-weight inputs and all output DMAs ride
the Scalar HWDGE queue (a separate ring the SDMA round-robins with), so
output traffic can never starve the weight prefetch and output staging
recycles promptly.

Features are pre-normalized on device (S/|f| folded into the moving operand).
The ArcFace margin values for every row are computed on device in a
replicated-row layout (no per-row gather needed) and written to a small vout
tensor; the host applies them to the target columns by fancy indexing
(indexing only, no host FLOPs).

Everything on device runs in bf16 (full PE rate), with all reductions
accumulating in fp32 PSUM.
"""

import math
import os

import numpy as np

B = 512
D = 512
C = 100000
NCORES = 8
CS = C // NCORES          # 12500 classes per core

M_MARGIN = 0.5
S_SCALE = 64.0
TH = math.cos(math.pi - M_MARGIN)
MM_ = math.sin(math.pi - M_MARGIN) * M_MARGIN

P = 128
NK = D // P               # 4 contraction chunks
NCH = (CS + P - 1) // P   # 98 class chunks (97*128 + 84)
PW = 1024                 # classes per piece (8 chunks)
NPC = (CS + PW - 1) // PW  # 13 pieces, last = 212 wide

_CACHE = {}


def _pieces():
    out = []
    for p in range(NPC):
        lo = p * PW
        hi = min(CS, lo + PW)
        out.append((lo, hi))
    return out


def _build_nc(opts=None):
    opts = opts or {}
    import concourse.tile as tile
    from concourse import bacc, mybir

    dt = mybir.dt
    Alu = mybir.AluOpType
    Act = mybir.ActivationFunctionType

    nc = bacc.Bacc("TRN2", target_bir_lowering=False, debug=False,
                   enable_asserts=False, num_devices=NCORES)

    wt = nc.dram_tensor("wt", [D, CS], dt.bfloat16, kind="ExternalInput").ap()
    featT = nc.dram_tensor("featT", [D, B], dt.bfloat16,
                           kind="ExternalInput").ap()
    wgT = nc.dram_tensor("wgT", [D, B], dt.bfloat16, kind="ExternalInput").ap()
    out = nc.dram_tensor("out", [CS, B], dt.bfloat16, kind="ExternalOutput").ap()
    vout = nc.dram_tensor("vout", [1, B], dt.float32,
                          kind="ExternalOutput").ap()

    ev_eng = opts.get("ev_eng", "aav")    # eviction engine cycle (per chunk)
    sq_eng = opts.get("sq_eng", "v")      # engine cycle for piece squares
    JB = opts.get("jb", 8)                # chunks batched per output DMA
    pieces = _pieces()

    with tile.TileContext(nc) as tc:
        with (
            tc.tile_pool(name="const", bufs=1) as constp,
            tc.tile_pool(name="fm", bufs=2) as fmp,      # feature/margin scratch
            tc.tile_pool(name="sqs", bufs=2) as sqp,     # square scratch
            tc.tile_pool(name="outp", bufs=opts.get("osb_bufs", 4)) as outp,
            tc.tile_pool(name="ps_o", bufs=opts.get("ps_o", 6),
                         space="PSUM") as ps_o,
            tc.tile_pool(name="ps_n", bufs=1, space="PSUM") as ps_n,
            tc.tile_pool(name="ps_f", bufs=1, space="PSUM") as ps_f,
        ):
            # ---- constants ----
            ones_sq = constp.tile([P, P], dt.bfloat16, tag="ones_sq")
            nc.vector.memset(ones_sq[:], 1.0)
            ones_col = constp.tile([P, 1], dt.bfloat16, tag="ones_col")
            nc.vector.memset(ones_col[:], 1.0)

            # ---- Act table preload: touch every activation function used
            # later so the ~1.5us ACT_TABLE_LOADs happen during boot idle ----
            actwarm = constp.tile([P, 2], dt.bfloat16, tag="actwarm")
            nc.scalar.activation(actwarm[:, 0:1], ones_col[:], Act.Square,
                                 bias=0.0, scale=1.0)
            nc.scalar.activation(actwarm[:, 1:2], ones_col[:],
                                 Act.Abs_reciprocal_sqrt, bias=0.0, scale=1.0)

            # ---- PE pstate warmup: dummy matmuls while DMAs land ----
            nwarm = opts.get("nwarm", 20)
            if nwarm:
                warm = ps_f.tile([P, B], dt.float32, tag="psf")
                for _ in range(nwarm):
                    nc.tensor.matmul(warm[:, 0:P], ones_sq[:], ones_sq[:],
                                     start=True, stop=True)

            # ---- input DMAs, all on the Sync HWDGE queue, issued up-front
            # with no interleaved waits. featT + piece 0 go first at per-k
            # granularity so the boot chain pipelines; later pieces are one
            # bundled 3D DMA each. ----
            fT = constp.tile([P, NK * B], dt.bfloat16, tag="fT")
            for k in range(NK):
                nc.sync.dma_start(fT[:, k * B:(k + 1) * B],
                                  featT[k * P:(k + 1) * P, :])

            # Weight pieces live in four per-k rotating 6-slot pools (one DMA
            # per (piece, k); separate pools keep the four stationary tiles
            # of a chunk far apart in SBUF, matching the address phasing the
            # PE LDWEIGHTS stream runs fastest with). DMAs run 4-5 pieces
            # ahead of consumption, so slot reuse never has to wait.
            wtps = {}

            def emit_wt_dma(p, halves=1):
                lo, hi = pieces[p]
                tiles = []
                for k in range(NK):
                    wtpk = sqp.tile([P, PW], dt.bfloat16, tag=f"wtp{k}",
                                    bufs=6)
                    tiles.append(wtpk)
                hw = (hi - lo) // halves
                for h in range(halves):
                    for k in range(NK):
                        nc.sync.dma_start(
                            tiles[k][:, h * hw:(h + 1) * hw],
                            wt[k * P:(k + 1) * P,
                               lo + h * hw:lo + (h + 1) * hw])
                wtps[p] = tiles

            for p in range(0, 5):
                emit_wt_dma(p)
            wG = constp.tile([P, NK * B], dt.bfloat16, tag="wG")
            nc.sync.dma_start(
                wG[:, :].rearrange("p (k b) -> p k b", k=NK),
                wgT[:, :].rearrange("(k p) b -> p k b", k=NK, p=P))

            # ---- feature normalization: fhat = S * f / |f|, bf16;
            # per-k so the chain pipelines with the fT chunk DMAs ----
            sqf = fmp.tile([P, NK * B], dt.bfloat16, tag="sqf", bufs=1)
            ssf = ps_f.tile([P, B], dt.float32, tag="psf")
            for k in range(NK):
                nc.vector.tensor_mul(sqf[:, k * B:(k + 1) * B],
                                     fT[:, k * B:(k + 1) * B],
                                     fT[:, k * B:(k + 1) * B])
            for k in range(NK):
                nc.tensor.matmul(ssf[:], ones_sq[:], sqf[:, k * B:(k + 1) * B],
                                 start=(k == 0), stop=(k == NK - 1))
            invfS = constp.tile([P, B], dt.bfloat16, tag="invfS")
            # rsqrt(ssf / S^2) = S / |f|
            nc.scalar.activation(invfS[:], ssf[:], Act.Abs_reciprocal_sqrt,
                                 bias=0.0, scale=1.0 / (S_SCALE * S_SCALE))
            fhat = constp.tile([P, NK * B], dt.bfloat16, tag="fhat")
            for k in range(NK):
                nc.vector.tensor_tensor(out=fhat[:, k * B:(k + 1) * B],
                                        in0=fT[:, k * B:(k + 1) * B],
                                        in1=invfS[:], op=Alu.mult)

            # ---- resident norm scale tile ----
            invw = constp.tile([P, NCH], dt.float32, tag="invw")

            def emit_chain(p):
                # squared weights + k-reduction for one piece
                lo, hi = pieces[p]
                w = hi - lo
                sq = sqp.tile([P, NK * PW], dt.bfloat16, tag="sq", bufs=2)
                t1 = sqp.tile([P, 2 * PW], dt.bfloat16, tag="t1", bufs=2)
                pp = sqp.tile([P, PW], dt.bfloat16, tag="pp", bufs=2)
                e = sq_eng[p % len(sq_eng)]
                for k in range(NK):
                    ksl = slice(k * w, (k + 1) * w)
                    src = wtps[p][k][:, 0:w]
                    if (e == "a") != (p == 0 and k % 2 == 1):
                        nc.scalar.activation(sq[:, ksl], src, Act.Square,
                                             bias=0.0, scale=1.0)
                    else:
                        nc.vector.tensor_mul(sq[:, ksl], src, src)
                nc.vector.tensor_tensor(out=t1[:, 0:2 * w],
                                        in0=sq[:, 0:2 * w],
                                        in1=sq[:, 2 * w:4 * w], op=Alu.add)
                nc.vector.tensor_tensor(out=pp[:, 0:w], in0=t1[:, 0:w],
                                        in1=t1[:, w:2 * w], op=Alu.add)
                return pp

            # chunk lists per piece: (global chunk id, col-in-piece, width)
            chunks_of = []
            gfirst = 0
            for (lo, hi) in pieces:
                glast = min(NCH, (hi + P - 1) // P)
                chunks_of.append([(g, g * P - lo, min(P, CS - g * P))
                                  for g in range(gfirst, glast)])
                gfirst = glast

            def emit_norms(p, pp):
                # class norms: one 1-col matmul per 128-class chunk, then the
                # per-partition inv-norm via Act rsqrt into resident invw
                chs = chunks_of[p]
                nch = len(chs)
                g0 = chs[0][0]
                nps = ps_n.tile([P, 8], dt.float32, tag="nps")
                for i, (g, c0, cw) in enumerate(chs):
                    nc.tensor.matmul(nps[0:cw, i:i + 1],
                                     pp[:, c0:c0 + cw], ones_col[:],
                                     start=True, stop=True)
                cwl = chs[-1][2]
                if cwl == P:
                    nc.scalar.activation(invw[:, g0:g0 + nch], nps[:, 0:nch],
                                         Act.Abs_reciprocal_sqrt,
                                         bias=0.0, scale=1.0)
                else:  # last chunk is 84 classes; avoid unwritten PSUM rows
                    if nch > 1:
                        nc.scalar.activation(invw[:, g0:g0 + nch - 1],
                                             nps[:, 0:nch - 1],
                                             Act.Abs_reciprocal_sqrt,
                                             bias=0.0, scale=1.0)
                    nc.scalar.activation(invw[0:cwl, g0 + nch - 1:g0 + nch],
                                         nps[0:cwl, nch - 1:nch],
                                         Act.Abs_reciprocal_sqrt,
                                         bias=0.0, scale=1.0)

            mtiles = {}

            def emit_margin_pre():
                # DVE products for the margin reductions, emitted well before
                # the margin matmuls so the tensor queue never waits on them
                sqwg = fmp.tile([P, NK * B], dt.bfloat16, tag="sqwg", bufs=1)
                nc.vector.tensor_mul(sqwg[:], wG[:], wG[:])
                fg = fmp.tile([P, NK * B], dt.bfloat16, tag="fg", bufs=1)
                nc.vector.tensor_mul(fg[:], fhat[:], wG[:])
                mtiles["sqwg"] = sqwg
                mtiles["fg"] = fg

            def emit_margin():
                """ArcFace margin: reductions + trig; inputs already in SBUF
                so the tensor-queue matmuls never stall."""
                sqwg = mtiles["sqwg"]
                fg = mtiles["fg"]
                sswg = ps_f.tile([P, B], dt.float32, tag="psf")
                for k in range(NK):
                    nc.tensor.matmul(sswg[:], ones_sq[:],
                                     sqwg[:, k * B:(k + 1) * B],
                                     start=(k == 0), stop=(k == NK - 1))
                # rsqrt(sswg * S^2) = 1 / (S * |wg|)
                invwg = fmp.tile([1, B], dt.bfloat16, tag="invwg", bufs=1)
                nc.scalar.activation(invwg[:], sswg[0:1, :],
                                     Act.Abs_reciprocal_sqrt,
                                     bias=0.0, scale=S_SCALE * S_SCALE)
                dot = ps_f.tile([P, B], dt.float32, tag="psf")
                for k in range(NK):
                    nc.tensor.matmul(dot[:], ones_sq[:],
                                     fg[:, k * B:(k + 1) * B],
                                     start=(k == 0), stop=(k == NK - 1))
                # t = cos(theta); the reductions are partition-replicated,
                # so the scalar chain runs on single-partition [1, B] slices
                t = fmp.tile([1, B], dt.bfloat16, tag="t", bufs=1)
                nc.vector.tensor_tensor(out=t[:], in0=dot[0:1, :],
                                        in1=invwg[:], op=Alu.mult)
                nc.vector.tensor_scalar_min(t[:], t[:], 1.0)
                nc.vector.tensor_scalar_max(t[:], t[:], -1.0)
                om = fmp.tile([1, B], dt.bfloat16, tag="om", bufs=1)
                nc.vector.tensor_mul(om[:], t[:], t[:])
                nc.vector.tensor_scalar(out=om[:], in0=om[:], scalar1=-1.0,
                                        scalar2=1.0, op0=Alu.mult, op1=Alu.add)
                rs = fmp.tile([1, B], dt.bfloat16, tag="rs", bufs=1)
                nc.scalar.activation(rs[:], om[:], Act.Abs_reciprocal_sqrt,
                                     bias=0.0, scale=1.0)
                r = fmp.tile([1, B], dt.bfloat16, tag="r", bufs=1)
                nc.vector.tensor_mul(r[:], om[:], rs[:])   # sqrt(1 - t^2)
                a1 = fmp.tile([1, B], dt.bfloat16, tag="a1", bufs=1)
                nc.vector.tensor_scalar_mul(a1[:], t[:], math.cos(M_MARGIN))
                a2 = fmp.tile([1, B], dt.bfloat16, tag="a2", bufs=1)
                nc.vector.tensor_scalar_mul(a2[:], r[:], math.sin(M_MARGIN))
                adjA = fmp.tile([1, B], dt.bfloat16, tag="adjA", bufs=1)
                nc.vector.tensor_tensor(out=adjA[:], in0=a1[:], in1=a2[:],
                                        op=Alu.subtract)
                mask = fmp.tile([1, B], dt.int8, tag="mask", bufs=1)
                nc.vector.tensor_scalar(out=mask[:], in0=t[:], scalar1=TH,
                                        scalar2=None, op0=Alu.is_gt)
                adj = fmp.tile([1, B], dt.bfloat16, tag="adj", bufs=1)
                nc.vector.tensor_scalar_sub(adj[:], t[:], MM_)
                nc.vector.copy_predicated(adj[:], mask[:], adjA[:])
                val = fmp.tile([1, B], dt.float32, tag="val", bufs=1)
                nc.vector.tensor_scalar_mul(val[:], adj[:], S_SCALE)
                nc.scalar.dma_start(vout[:], val[0:1, :])

            def emit_batch_mms(p, i, nb):
                # PE matmuls for one output batch of nb chunks
                lo, hi = pieces[p]
                w = hi - lo
                chs = chunks_of[p]
                wtp = wtps[p]
                pos = []
                for j in range(nb):
                    g, c0, cw = chs[i + j]
                    po = ps_o.tile([P, B], dt.float32, tag="po")
                    for k in range(NK):
                        nc.tensor.matmul(
                            po[0:cw, :],
                            wtp[k][:, c0:c0 + cw],
                            fhat[:, k * B:(k + 1) * B],
                            start=(k == 0), stop=(k == NK - 1))
                    pos.append(po)
                return pos

            def emit_batch_out(p, i, pos):
                # evictions (scale-copy, spread over Act/DVE/GpSimd per
                # ev_eng) + one batched out DMA on the Scalar HWDGE queue
                lo, hi = pieces[p]
                chs = chunks_of[p]
                nb = len(pos)
                full = all(chs[i + j][2] == P for j in range(nb))
                osb = outp.tile([P, nb * B], dt.bfloat16, tag=f"osb{nb}",
                                bufs=opts.get("osb_bufs", 6) if nb == JB
                                else 2)
                for j in range(nb):
                    g, c0, cw = chs[i + j]
                    osl = osb[0:cw, j * B:j * B + B]
                    # the final piece's two evictions run on different
                    # engines so they finish in parallel at the tail
                    e = ("av"[j % 2] if p == NPC - 1
                         else ev_eng[g % len(ev_eng)])
                    if e == "v":
                        nc.vector.tensor_scalar_mul(osl, pos[j][0:cw, :],
                                                    invw[0:cw, g:g + 1])
                    else:
                        nc.scalar.activation(osl, pos[j][0:cw, :], Act.Copy,
                                             bias=0.0,
                                             scale=invw[0:cw, g:g + 1])
                c0 = chs[i][1] + lo
                # the last (small) piece drains on the otherwise-idle Scalar
                # HWDGE ring; everything else on Sync. Keeping the big
                # second-to-last batches off the Act queue matters: a 2.4us
                # DMA issue there would delay the final evictions
                deng = nc.scalar if p == NPC - 1 else nc.sync
                if full:
                    dst = out[c0:c0 + nb * P, :].rearrange(
                        "(j p) b -> p j b", j=nb, p=P)
                    src = osb[:, 0:nb * B].rearrange("p (j b) -> p j b", j=nb)
                    deng.dma_start(dst, src)
                else:
                    for j in range(nb):
                        g, cj, cw = chs[i + j]
                        dj = nc.scalar if (p == NPC - 1 and j % 2 == 0) \
                            else nc.sync
                        dj.dma_start(
                            out[lo + cj:lo + cj + cw, :],
                            osb[0:cw, j * B:j * B + B])

            def emit_mains(p, jb=JB, i0=0):
                chs = chunks_of[p]
                i = i0
                while i < len(chs):
                    nb = min(jb, len(chs) - i)
                    pos = emit_batch_mms(p, i, nb)
                    emit_batch_out(p, i, pos)
                    i += nb

            # ---- software pipeline over class pieces ----
            # Piece 0 is special: the first 4 chunks' matmuls are emitted
            # ahead of the norm matmuls (PE starts as soon as fhat + piece 0
            # are in SBUF) while their evictions queue behind the invw rsqrt.
            pps = {0: emit_chain(0), 1: emit_chain(1)}
            pos0 = emit_batch_mms(0, 0, 4)
            emit_norms(0, pps[0])
            emit_batch_out(0, 0, pos0)
            emit_mains(0, i0=4)
            emit_norms(1, pps[1])
            for p in range(1, NPC):
                if p + 4 < NPC:
                    emit_wt_dma(p + 4)
                if p + 1 < NPC:
                    pps[p + 1] = emit_chain(p + 1)
                if p == 1:
                    emit_margin_pre()
                if p == 4:
                    emit_margin()
                # the last full piece drains in 4-chunk batches so its out
                # DMAs overlap the final evictions instead of one big issue
                emit_mains(p, jb=4 if p == NPC - 2 else JB)
                if p + 1 < NPC:
                    emit_norms(p + 1, pps[p + 1])

    nc.compile()
    return nc


def _get_nc(opts=None):
    key = tuple(sorted((opts or {}).items()))
    if key not in _CACHE:
        _CACHE[key] = _build_nc(opts)
    return _CACHE[key]


def _enable_trace_hook():
    import sys
    import types
    try:
        import antenv.axon_hooks  # noqa: F401
        return
    except ImportError:
        pass
    import antenv
    mod = types.ModuleType("antenv.axon_hooks")
    holder = [None]
    mod.set_axon_ntff_profile_hook = lambda h: holder.__setitem__(0, h)
    mod.get_axon_ntff_profile_hook = lambda: holder[0]
    sys.modules["antenv.axon_hooks"] = mod
    antenv.axon_hooks = mod
    try:
        from trn_agent_boot.trn_boot import _ntff_profile_via_ctypes
        mod.set_axon_ntff_profile_hook(
            _ntff_profile_via_ctypes("/opt/axon/libaxon_pjrt.so"))
    except Exception:
        pass


LAST_EXEC_NS = None
LAST_RESULTS = None
_OPTS = {}


def kernel(features, labels, weight):
    global LAST_EXEC_NS, LAST_RESULTS
    import ml_dtypes
    from concourse.bass_utils import run_bass_kernel_spmd

    features = np.asarray(features)
    weight = np.asarray(weight)
    labels = np.asarray(labels).astype(np.int64)

    trace = bool(int(os.environ.get("ARCFACE_TRACE", "0")))
    if trace:
        _enable_trace_hook()

    nc = _get_nc(_OPTS.get("opts"))

    featT_np = np.ascontiguousarray(features.T.astype(ml_dtypes.bfloat16))
    wgT_np = np.ascontiguousarray(weight[labels].T.astype(ml_dtypes.bfloat16))
    wt16 = weight.astype(ml_dtypes.bfloat16)

    in_maps = []
    for c in range(NCORES):
        c0 = c * CS
        wt_c = np.ascontiguousarray(wt16[c0:c0 + CS].T)  # [D, CS] bf16
        in_maps.append({
            "wt": wt_c,
            "featT": featT_np,
            "wgT": wgT_np,
        })

    res = run_bass_kernel_spmd(nc, in_maps, core_ids=list(range(NCORES)),
                               trace=trace)
    LAST_EXEC_NS = res.exec_time_ns
    LAST_RESULTS = res

    full = np.empty((B, C), dtype=np.float32)
    for c in range(NCORES):
        full[:, c * CS:(c + 1) * CS] = res.results[c]["out"].T
    rows = np.arange(B)
    full[rows, labels] = np.asarray(res.results[0]["vout"]).reshape(B)
    return full
